# revision 1
# baseline (speedup 1.0000x reference)
"""Distributed GQA attention kernel for Trainium2 (8 NeuronCores).

Module: B=4, S=2048, H=576, 9 Q heads / 3 KV heads, HD=64, RoPE, causal
softmax, output projection.

Sharding: core c handles batch c//2 and four 256-row query blocks
({0,3,4,7} for even c, {1,2,5,6} for odd c) -- causal work is balanced at
18 key-tile units per core. Every core computes its batch's full K/V
projection locally (duplicated across the 2 cores of a batch; cheaper
than an all-gather). One SPMD graph for all 8 cores: per-slot key-tile
extents are padded to [4,8,12,16] and the causal mask is applied from
per-core mask DATA on the last 4 key-tiles of each slot.

Everything lives transposed ([dim, seq]) so scoresT[k,q] chains
QK -> exp -> PV -> Wo with no on-chip transposes. Softmax skips the max
subtraction (scores are O(1) here); row-sums ride along as a 65th output
row of the PV matmul via a ones-column appended to V. Matmuls run in
float32r (full PE rate at free-dim >= 256).
"""

import sys

if "/opt/trn_rl_repo" not in sys.path:
    sys.path.insert(0, "/opt/trn_rl_repo")

import numpy as np

B, S, H = 4, 2048, 576
NH, NKV, HD = 9, 3, 64
GROUPS = NH // NKV  # 3 q heads per kv head
BLK = 256           # query block rows
NBLK = S // BLK     # 8
KT = 128            # key tile rows
EXT = [4, 8, 12, 16]  # padded key-tile extent per block slot
NMASK = 16          # total masked key-tiles per core (= sum of last-4 per slot)
BLOCKS_EVEN = [0, 3, 4, 7]
BLOCKS_ODD = [1, 2, 5, 6]
HK = [128, 128, 128, 128, 64]  # contraction tiles over H=576

_CACHED = {}


USE_BF16 = False


def _build(reps=1):
    from concourse import bacc, bass, mybir, tile

    f32 = mybir.dt.float32
    f32r = mybir.dt.bfloat16 if USE_BF16 else mybir.dt.float32r
    AF = mybir.ActivationFunctionType
    ALU = mybir.AluOpType

    nc = bacc.Bacc("TRN2", target_bir_lowering=False, debug=False)

    # ---- per-core inputs ----
    xT = nc.dram_tensor("xT", [H, S], f32r, kind="ExternalInput")
    xTq = nc.dram_tensor("xTq", [H, 4 * BLK], f32r, kind="ExternalInput")
    Wq = nc.dram_tensor("Wq", [H, NH * HD], f32r, kind="ExternalInput")
    Wk = nc.dram_tensor("Wk", [H, NKV * HD], f32r, kind="ExternalInput")
    Wvp = nc.dram_tensor("Wvp", [H, 256], f32r, kind="ExternalInput")  # Wv zero-padded
    Wo = nc.dram_tensor("Wo", [NH * HD, H], f32r, kind="ExternalInput")
    P = nc.dram_tensor("P", [HD, HD], f32r, kind="ExternalInput")  # rotate_half perm
    cosk = nc.dram_tensor("cosk", [HD, S], f32r, kind="ExternalInput")
    sink = nc.dram_tensor("sink", [HD, S], f32r, kind="ExternalInput")
    cosq = nc.dram_tensor("cosq", [HD, 4 * BLK], f32r, kind="ExternalInput")  # /8
    sinq = nc.dram_tensor("sinq", [HD, 4 * BLK], f32r, kind="ExternalInput")  # /8
    maskst = nc.dram_tensor("maskst", [NMASK, KT, BLK], f32r, kind="ExternalInput")
    ones16 = nc.dram_tensor("ones16", [128, 16], f32r, kind="ExternalInput")
    out = nc.dram_tensor("out", [4 * BLK, H], f32, kind="ExternalOutput")

    with tile.TileContext(nc) as tc:
        with (
            tc.tile_pool(name="consts", bufs=1) as cp,
            tc.tile_pool(name="xstream", bufs=2) as xsp,
            tc.tile_pool(name="kvres", bufs=1) as kvres,
            tc.tile_pool(name="qtp", bufs=1) as qtp,
            tc.tile_pool(name="work", bufs=2) as wp,
            tc.tile_pool(name="expp", bufs=4) as expp,
            tc.tile_pool(name="mskp", bufs=1) as mskp,
            tc.tile_pool(name="ctp", bufs=1) as ctp,
            tc.tile_pool(name="outp", bufs=2) as outp,
            tc.tile_pool(name="ps1", bufs=2, space="PSUM") as ps1,
            tc.tile_pool(name="scp", bufs=2, space="PSUM") as scp,
            tc.tile_pool(name="psA", bufs=1, space="PSUM") as psA,
        ):
            # ---- load constants ----
            def load_w(dram, cols, defer=False):
                tiles = []
                r0 = 0
                for hk in HK:
                    t = cp.tile([hk, cols], f32r, tag=f"w{dram.name}{r0}", name=f"w{dram.name}{r0}")
                    if not defer:
                        nc.sync.dma_start(t[:], dram.ap()[r0 : r0 + hk, :])
                    tiles.append(t)
                    r0 += hk
                return tiles

            Wk_sb = load_w(Wk, NKV * HD)
            Wvp_sb = load_w(Wvp, 256)
            Wq_sb = load_w(Wq, NH * HD)
            Wo_sb = load_w(Wo, H)
            P_sb = cp.tile([HD, HD], f32r, tag="P")
            nc.sync.dma_start(P_sb[:], P.ap())
            cosq_sb = cp.tile([HD, 4 * BLK], f32r, tag="cosq")
            sinq_sb = cp.tile([HD, 4 * BLK], f32r, tag="sinq")
            for t, d in ((cosq_sb, cosq), (sinq_sb, sinq)):
                nc.sync.dma_start(t[:], d.ap())
            ones_sb = cp.tile([128, 16], f32r, tag="ones")
            nc.sync.dma_start(ones_sb[:], ones16.ap())
            # trigger the exp ACT-table load during the startup DMA wait
            warm = cp.tile([1, 1], f32, tag="warm")
            nc.scalar.activation(warm[:], ones_sb[0:1, 0:1], AF.Exp)

            def one_pass():
                # ---- K/V projection, streaming xT in 512-token chunks ----
                # (replicated `reps` times for slope timing; reps=1 in production)
                kTp = [
                    [kvres.tile([HD, 512], f32r, tag=f"kT{g}c{ch}", name=f"kTp{g}c{ch}") for ch in range(4)]
                    for g in range(NKV)
                ]
                v_aug = [
                    [kvres.tile([128, 4 * 65], f32r, tag=f"vaug{g}c{ch}", name=f"vaug{g}c{ch}") for ch in range(4)]
                    for g in range(NKV)
                ]
                for ch in range(4):
                    c0 = ch * 512
                    xch = []
                    r0 = 0
                    for kt, hk in enumerate(HK):
                        t = xsp.tile([hk, 512], f32r, tag=f"xch{kt}", name=f"xch{kt}")
                        nc.sync.dma_start(t[:], xT.ap()[r0 : r0 + hk, c0 : c0 + 512])
                        xch.append(t)
                        r0 += hk
                    cosk_ch = xsp.tile([HD, 512], f32r, tag="coskch", name="cosk_ch")
                    nc.sync.dma_start(cosk_ch[:], cosk.ap()[:, c0 : c0 + 512])
                    sink_ch = xsp.tile([HD, 512], f32r, tag="sinkch", name="sink_ch")
                    nc.sync.dma_start(sink_ch[:], sink.ap()[:, c0 : c0 + 512])
                    for g in range(NKV):
                        kps = ps1.tile([HD, 512], f32, tag="ps1")
                        for kt in range(5):
                            nc.tensor.matmul(
                                kps[:],
                                Wk_sb[kt][:, g * HD : (g + 1) * HD],
                                xch[kt][:],
                                start=(kt == 0),
                                stop=(kt == 4),
                            )
                        kraw = wp.tile([HD, 512], f32r, tag="kraw")
                        nc.vector.tensor_copy(kraw[:], kps[:])
                        rps = ps1.tile([HD, 512], f32, tag="ps1")
                        nc.tensor.matmul(rps[:], P_sb[:], kraw[:], start=True, stop=True)
                        t1 = wp.tile([HD, 512], f32r, tag="t1")
                        nc.vector.tensor_tensor(t1[:], kraw[:], cosk_ch[:], ALU.mult)
                        t2 = wp.tile([HD, 512], f32r, tag="t2")
                        nc.vector.tensor_tensor(t2[:], rps[:], sink_ch[:], ALU.mult)
                        nc.vector.tensor_tensor(kTp[g][ch][:], t1[:], t2[:], ALU.add)
                    for st4 in range(4):
                        st = ch * 4 + st4
                        vps = ps1.tile([128, 256], f32, tag="ps1")
                        for kt in range(5):
                            nc.tensor.matmul(
                                vps[:],
                                xch[kt][:, st4 * 128 : (st4 + 1) * 128],
                                Wvp_sb[kt][:],
                                start=(kt == 0),
                                stop=(kt == 4),
                            )
                        for g in range(NKV):
                            nc.vector.tensor_copy(
                                v_aug[g][ch][:, st4 * 65 : st4 * 65 + 64],
                                vps[:, g * HD : (g + 1) * HD],
                            )
                for g in range(NKV):
                    for ch in range(4):
                        dst = v_aug[g][ch][:].rearrange("p (n c) -> p n c", c=65)[:, :, 64:65]
                        nc.vector.tensor_copy(dst, ones_sb[:, 0:4].unsqueeze(2))

                # ---- Q proj + RoPE for all 4 blocks at once ----
                xq = []
                r0 = 0
                for kt, hk in enumerate(HK):
                    t = qtp.tile([hk, 4 * BLK], f32r, tag=f"xq{kt}", name=f"xq{kt}")
                    nc.sync.dma_start(t[:], xTq.ap()[r0 : r0 + hk, :])
                    xq.append(t)
                    r0 += hk
                qTall = qtp.tile([HD, NH * 4 * BLK], f32r, tag="qTall", name="qTall")
                # layout: [64, h*1024 + blk*256]
                for h in range(NH):
                    for cc in range(2):  # two 512-col chunks of the 1024 q cols
                        qps = ps1.tile([HD, 512], f32, tag="ps1")
                        for kt in range(5):
                            nc.tensor.matmul(
                                qps[:],
                                Wq_sb[kt][:, h * HD : (h + 1) * HD],
                                xq[kt][:, cc * 512 : (cc + 1) * 512],
                                start=(kt == 0),
                                stop=(kt == 4),
                            )
                        qraw = wp.tile([HD, 512], f32r, tag="kraw")
                        nc.vector.tensor_copy(qraw[:], qps[:])
                        rps = ps1.tile([HD, 512], f32, tag="ps1")
                        nc.tensor.matmul(rps[:], P_sb[:], qraw[:], start=True, stop=True)
                        tq1 = wp.tile([HD, 512], f32r, tag="t1")
                        nc.vector.tensor_tensor(
                            tq1[:], qraw[:], cosq_sb[:, cc * 512 : (cc + 1) * 512], ALU.mult
                        )
                        tq2 = wp.tile([HD, 512], f32r, tag="t2")
                        nc.vector.tensor_tensor(
                            tq2[:], rps[:], sinq_sb[:, cc * 512 : (cc + 1) * 512], ALU.mult
                        )
                        nc.vector.tensor_tensor(
                            qTall[:, h * 1024 + cc * 512 : h * 1024 + (cc + 1) * 512],
                            tq1[:],
                            tq2[:],
                            ALU.add,
                        )

                for j in range(4):
                    q0 = j * BLK
                    qv = qTall[:].rearrange("p (h j c) -> p h j c", j=4, c=BLK)
                    # concat tiles for Wo lhsT: heads 2t (rows 0:64), 2t+1 (rows 64:128)
                    cts = [ctp.tile([128, BLK], f32r, tag=f"ct{t}", name=f"ct{t}") for t in range(4)]
                    cts.append(ctp.tile([HD, BLK], f32r, tag="ct4", name="ct4"))

                    ext = EXT[j]
                    mts = {}
                    for off in range(4):
                        kcm = ext - 4 + off
                        mt = mskp.tile([KT, BLK], f32r, tag=f"msk{off}", name=f"msk{off}")
                        nc.sync.dma_start(mt[:], maskst.ap()[kcm, :, :])
                        mts[kcm] = mt
                    for g in range(NKV):
                        h0 = 3 * g
                        accp = psA.tile([65, 512], f32, tag="accp", name="accp")
                        accs1 = psA.tile([65, BLK], f32, tag="accs", name="accs")
                        for kc in range(ext):
                            masked = kc >= ext - 4
                            if masked:
                                mt = mts[kc]
                            sps = scp.tile([KT, 3 * BLK], f32, tag="sc")
                            nc.tensor.matmul(
                                sps[:, 0:512],
                                kTp[g][kc // 4][:, (kc % 4) * KT : (kc % 4 + 1) * KT],
                                qv[:, h0 : h0 + 2, j, :],
                                start=True,
                                stop=True,
                            )
                            nc.tensor.matmul(
                                sps[:, 512:768],
                                kTp[g][kc // 4][:, (kc % 4) * KT : (kc % 4 + 1) * KT],
                                qv[:, h0 + 2 : h0 + 3, j, :],
                                start=True,
                                stop=True,
                            )
                            esb = expp.tile([KT, 3 * BLK], f32r, tag="exp")
                            nc.scalar.activation(esb[:], sps[:], AF.Exp)
                            if masked:
                                for i in range(3):
                                    sl = esb[:, i * BLK : (i + 1) * BLK]
                                    nc.gpsimd.tensor_tensor(sl, sl, mt[:], ALU.mult)
                            nc.tensor.matmul(
                                accp[:],
                                v_aug[g][kc // 4][:, (kc % 4) * 65 : (kc % 4) * 65 + 65],
                                esb[:, 0:512],
                                start=(kc == 0),
                                stop=(kc == ext - 1),
                            )
                            nc.tensor.matmul(
                                accs1[:],
                                v_aug[g][kc // 4][:, (kc % 4) * 65 : (kc % 4) * 65 + 65],
                                esb[:, 512:768],
                                start=(kc == 0),
                                stop=(kc == ext - 1),
                            )
                        for acc, width, heads in (
                            (accp, 512, (h0, h0 + 1)),
                            (accs1, 256, (h0 + 2,)),
                        ):
                            rec = wp.tile([128, 512], f32, tag="rec")
                            nc.vector.reciprocal(rec[64:65, 0:width], acc[64:65, 0:width])
                            nc.sync.dma_start(rec[0:1, 0:width], rec[64:65, 0:width])
                            bc = wp.tile([HD, 512], f32, tag="bc")
                            nc.gpsimd.partition_broadcast(bc[:, 0:width], rec[0:1, 0:width])
                            for i, h in enumerate(heads):
                                c0 = i * BLK
                                t, lo = divmod(h, 2)
                                if lo == 0:
                                    nc.vector.tensor_tensor(
                                        cts[t][0:HD, :],
                                        acc[0:HD, c0 : c0 + BLK],
                                        bc[:, c0 : c0 + BLK],
                                        ALU.mult,
                                    )
                                else:
                                    stg = wp.tile([HD, BLK], f32r, tag="stg")
                                    nc.vector.tensor_tensor(
                                        stg[:],
                                        acc[0:HD, c0 : c0 + BLK],
                                        bc[:, c0 : c0 + BLK],
                                        ALU.mult,
                                    )
                                    nc.sync.dma_start(cts[t][HD:128, :], stg[:])

                    # out projection: out[q, :] = sum_t cts[t][:, q].T @ Wo_sb[t]
                    for half in range(2):
                        h0 = half * 128
                        pa = ps1.tile([128, 512], f32, tag="ps1")
                        pb = ps1.tile([128, 64], f32, tag="ps1")
                        for t in range(5):
                            lhsT = cts[t][:, h0 : h0 + 128]
                            nc.tensor.matmul(
                                pa[:], lhsT, Wo_sb[t][:, 0:512], start=(t == 0), stop=(t == 4)
                            )
                            nc.tensor.matmul(
                                pb[:], lhsT, Wo_sb[t][:, 512:576], start=(t == 0), stop=(t == 4)
                            )
                        osb = outp.tile([128, H], f32, tag="osb")
                        nc.vector.tensor_copy(osb[:, 0:512], pa[:])
                        nc.vector.tensor_copy(osb[:, 512:576], pb[:])
                        nc.sync.dma_start(out.ap()[q0 + h0 : q0 + h0 + 128, :], osb[:])

            for _rep in range(reps):
                one_pass()

    nc.compile()
    return nc


def _get_nc(reps=1):
    key = f"nc{reps}"
    if key not in _CACHED:
        _CACHED[key] = _build(reps=reps)
    return _CACHED[key]


def _make_in_maps(x, cos, sin, mask, Wq, Wk, Wv, Wo):
    f4 = np.float32
    if USE_BF16:
        import ml_dtypes

        dtc = ml_dtypes.bfloat16
    else:
        dtc = np.float32
    Wvp = np.zeros((H, 256), f4)
    Wvp[:, : NKV * HD] = Wv
    P = np.zeros((HD, HD), f4)
    half = HD // 2
    for m in range(half):
        P[m + half, m] = -1.0
    for m in range(half, HD):
        P[m - half, m] = 1.0
    cosT = np.ascontiguousarray(cos.T.astype(f4))  # [64, S]
    sinT = np.ascontiguousarray(sin.T.astype(f4))
    scale = np.float32(1.0 / np.sqrt(HD))
    maskT_full = np.ascontiguousarray(mask[0, 0].T.astype(f4))  # [k, q]
    ones16 = np.ones((128, 16), f4)

    in_maps = []
    for c in range(8):
        b = c // 2
        blocks = BLOCKS_EVEN if c % 2 == 0 else BLOCKS_ODD
        xb = x[b]  # [S, H]
        xTc = np.ascontiguousarray(xb.T.astype(f4))  # [H, S]
        qcols = np.concatenate([xTc[:, blk * BLK : (blk + 1) * BLK] for blk in blocks], axis=1)
        cosq = np.concatenate(
            [cosT[:, blk * BLK : (blk + 1) * BLK] for blk in blocks], axis=1
        ) * scale
        sinq = np.concatenate(
            [sinT[:, blk * BLK : (blk + 1) * BLK] for blk in blocks], axis=1
        ) * scale
        maskstk = np.empty((NMASK, KT, BLK), f4)
        for j, blk in enumerate(blocks):
            for off in range(4):
                kc = 4 * j + off
                sl = maskT_full[kc * KT : (kc + 1) * KT, blk * BLK : (blk + 1) * BLK]
                maskstk[kc] = (sl > -1.0).astype(f4)
        in_maps.append(
            {
                "xT": xTc.astype(dtc),
                "xTq": np.ascontiguousarray(qcols).astype(dtc),
                "Wq": Wq.astype(f4).astype(dtc),
                "Wk": Wk.astype(f4).astype(dtc),
                "Wvp": Wvp.astype(dtc),
                "Wo": Wo.astype(f4).astype(dtc),
                "P": P.astype(dtc),
                "cosk": cosT.astype(dtc),
                "sink": sinT.astype(dtc),
                "cosq": np.ascontiguousarray(cosq).astype(dtc),
                "sinq": np.ascontiguousarray(sinq).astype(dtc),
                "maskst": maskstk.astype(dtc),
                "ones16": ones16.astype(dtc),
            }
        )
    return in_maps


def kernel(x, cos, sin, mask, Wq, Wk, Wv, Wo, _trace=False, _trace_kwargs=None):
    from concourse import bass_utils

    x = np.asarray(x)
    in_maps = _make_in_maps(
        np.asarray(x), np.asarray(cos), np.asarray(sin), np.asarray(mask),
        np.asarray(Wq), np.asarray(Wk), np.asarray(Wv), np.asarray(Wo),
    )
    nc = _get_nc()
    kw = {}
    if _trace:
        kw["trace"] = True
        if _trace_kwargs:
            kw.update(_trace_kwargs)
    res = bass_utils.run_bass_kernel_spmd(nc, in_maps, core_ids=list(range(8)), **kw)
    out = np.empty((B, S, H), np.float32)
    for c in range(8):
        b = c // 2
        blocks = BLOCKS_EVEN if c % 2 == 0 else BLOCKS_ODD
        o = res.results[c]["out"]  # [1024, 576]
        for j, blk in enumerate(blocks):
            out[b, blk * BLK : (blk + 1) * BLK, :] = o[j * BLK : (j + 1) * BLK, :]
    if _trace:
        _CACHED["last_result"] = res
    return out



# revision 5
# speedup vs baseline: 1.7554x; 1.7554x over previous
"""Distributed GQA attention kernel for Trainium2 (8 NeuronCores).

Module: B=4, S=2048, H=576, 9 Q heads / 3 KV heads, HD=64, RoPE, causal
softmax, output projection.

Sharding: core c handles batch c//2 and four 256-row query blocks
({0,3,4,7} for even c, {1,2,5,6} for odd c) -- causal work is balanced at
18 key-tile units per core. Every core computes its batch's full K/V
projection locally (duplicated across the 2 cores of a batch; cheaper
than an all-gather). One SPMD graph for all 8 cores: per-slot key-tile
extents are padded to [4,8,12,16] and the causal mask is applied from
per-core mask DATA on the last 4 key-tiles of each slot.

v2: bf16 on SBUF throughout (f32 PSUM accumulate), head-pairs packed
into 128 partitions for proj+RoPE, single [65,768] PV accumulator with
one reciprocal+broadcast per (block,group), no SBUF->SBUF DMAs
(cross-partition DVE writes), Wv stored [H,195] so PV lhsT slices need
no per-group copies, output staged bf16.
"""

import sys

if "/opt/trn_rl_repo" not in sys.path:
    sys.path.insert(0, "/opt/trn_rl_repo")

import numpy as np

B, S, H = 4, 2048, 576
NH, NKV, HD = 9, 3, 64
BLK = 256           # query block rows
KT = 128            # key tile rows
EXT = [4, 8, 12, 16]  # padded key-tile extent per block slot
BLOCKS_EVEN = [0, 3, 4, 7]
BLOCKS_ODD = [1, 2, 5, 6]
HK = [128, 128, 128, 128, 64]  # contraction tiles over H=576
CH = 1024           # kv chunk width (tokens)

_CACHED = {}


def _build(reps=1):
    from concourse import bacc, bass, mybir, tile

    f32 = mybir.dt.float32
    bf16 = mybir.dt.bfloat16
    AF = mybir.ActivationFunctionType
    ALU = mybir.AluOpType

    nc = bacc.Bacc("TRN2", target_bir_lowering=False, debug=False)

    # ---- per-core inputs (bf16 unless noted) ----
    xT = nc.dram_tensor("xT", [H, S], bf16, kind="ExternalInput")
    xTq = nc.dram_tensor("xTq", [H, 4 * BLK], bf16, kind="ExternalInput")
    Wq = nc.dram_tensor("Wq", [H, NH * HD], bf16, kind="ExternalInput")   # 1/8 folded
    Wk = nc.dram_tensor("Wk", [H, NKV * HD], bf16, kind="ExternalInput")
    Wv65 = nc.dram_tensor("Wv65", [H, 3 * 65], bf16, kind="ExternalInput")
    Wo = nc.dram_tensor("Wo", [NH * HD, H], bf16, kind="ExternalInput")
    P2 = nc.dram_tensor("P2", [128, 128], bf16, kind="ExternalInput")  # blockdiag rot
    cosk = nc.dram_tensor("cosk", [HD, S], bf16, kind="ExternalInput")
    sink = nc.dram_tensor("sink", [HD, S], bf16, kind="ExternalInput")
    cosq = nc.dram_tensor("cosq", [HD, 4 * BLK], bf16, kind="ExternalInput")
    sinq = nc.dram_tensor("sinq", [HD, 4 * BLK], bf16, kind="ExternalInput")
    maskst = nc.dram_tensor("maskst", [4, KT, 4 * BLK], bf16, kind="ExternalInput")
    ones48 = nc.dram_tensor("ones48", [128, 48], bf16, kind="ExternalInput")
    out = nc.dram_tensor("out", [4 * BLK, H], bf16, kind="ExternalOutput")

    with tile.TileContext(nc) as tc:
        with (
            tc.tile_pool(name="consts", bufs=1) as cp,
            tc.tile_pool(name="xstream", bufs=2) as xsp,
            tc.tile_pool(name="kvres", bufs=1) as kvres,
            tc.tile_pool(name="qtp", bufs=1) as qtp,
            tc.tile_pool(name="work", bufs=2) as wp,
            tc.tile_pool(name="expp", bufs=4) as expp,
            tc.tile_pool(name="mskp", bufs=2) as mskp,
            tc.tile_pool(name="ctp", bufs=1) as ctp,
            tc.tile_pool(name="outp", bufs=2) as outp,
            tc.tile_pool(name="scp", bufs=2, space="PSUM") as scp,
            tc.tile_pool(name="acp", bufs=2, space="PSUM") as acp,
        ):
            # ---- load constants ----
            def load_w(dram, cols):
                tiles = []
                r0 = 0
                for kt, hk in enumerate(HK):
                    t = cp.tile([hk, cols], bf16, tag=f"w{dram.name}{r0}",
                                name=f"w{dram.name}{r0}")
                    nc.sync.dma_start(t[:], dram.ap()[r0 : r0 + hk, :])
                    tiles.append(t)
                    r0 += hk
                return tiles

            Wk_sb = load_w(Wk, NKV * HD)
            Wv_sb = load_w(Wv65, 3 * 65)
            Wq_sb = load_w(Wq, NH * HD)
            Wo_sb = load_w(Wo, H)
            P2_sb = cp.tile([128, 128], bf16, tag="P2")
            nc.sync.dma_start(P2_sb[:], P2.ap())
            # stacked-pair cos/sin (same 64 rows twice)
            cos2k = cp.tile([128, S], bf16, tag="cos2k")
            sin2k = cp.tile([128, S], bf16, tag="sin2k")
            cos2q = cp.tile([128, 4 * BLK], bf16, tag="cos2q")
            sin2q = cp.tile([128, 4 * BLK], bf16, tag="sin2q")
            for t, d in ((cos2k, cosk), (sin2k, sink), (cos2q, cosq), (sin2q, sinq)):
                nc.sync.dma_start(t[0:64, :], d.ap())
                nc.sync.dma_start(t[64:128, :], d.ap())
            ones_sb = cp.tile([128, 48], bf16, tag="ones")
            nc.sync.dma_start(ones_sb[:], ones48.ap())
            # trigger the exp ACT-table load during the startup DMA wait
            warm = cp.tile([1, 1], f32, tag="warm")
            nc.scalar.activation(warm[:], ones_sb[0:1, 0:1], AF.Exp)

            def one_pass():
                # K layout: pair tile [128, S] = heads (g0,g1) stacked, single [64, S]
                kTp = kvres.tile([128, S], bf16, tag="kTp", name="kTp")
                kTs = kvres.tile([HD, S], bf16, tag="kTs", name="kTs")
                # V+ones layout: [128, g*1040 + st*65 + c] (3 groups x 16 subtiles x 65)
                v_aug = kvres.tile([128, 3 * 16 * 65], bf16, tag="vaug", name="v_aug")

                # ---- K/V projection, streaming xT in 1024-token chunks ----
                for ch in range(2):
                    c0 = ch * CH
                    xch = []
                    r0 = 0
                    for kt, hk in enumerate(HK):
                        t = xsp.tile([hk, CH], bf16, tag=f"xch{kt}", name=f"xch{kt}")
                        nc.sync.dma_start(t[:], xT.ap()[r0 : r0 + hk, c0 : c0 + CH])
                        xch.append(t)
                        r0 += hk
                    # K pair (g0,g1): stationary Wk[:, 0:128]
                    kp = scp.tile([128, CH], f32, tag="sc", name="kp")
                    for hf in range(2):
                        for kt in range(5):
                            nc.tensor.matmul(
                                kp[:, hf * 512 : (hf + 1) * 512],
                                Wk_sb[kt][:, 0:128],
                                xch[kt][:, hf * 512 : (hf + 1) * 512],
                                start=(kt == 0),
                                stop=(kt == 4),
                            )
                    # K single (g2): stationary Wk[:, 128:192]
                    ks = acp.tile([HD, CH], f32, tag="acc", name="ks")
                    for hf in range(2):
                        for kt in range(5):
                            nc.tensor.matmul(
                                ks[:, hf * 512 : (hf + 1) * 512],
                                Wk_sb[kt][:, 128:192],
                                xch[kt][:, hf * 512 : (hf + 1) * 512],
                                start=(kt == 0),
                                stop=(kt == 4),
                            )

                    def rope(ps, pp, rows, cos_sb, sin_sb, co, w, outs):
                        """ps: PSUM proj [rows, w]; outs: list of (dst_ap) for the
                        final add (may be strided); pp: P2 slice rows."""
                        raw = wp.tile([rows, w], bf16, tag="raw", name="raw")
                        nc.vector.tensor_copy(raw[:], ps)
                        rot = scp.tile([rows, w], f32, tag="sc", name="rot")
                        for hf in range(0, w, 512):
                            nc.tensor.matmul(
                                rot[:, hf : hf + 512],
                                P2_sb[0:rows, 0:rows],
                                raw[:, hf : hf + 512],
                                start=True,
                                stop=True,
                            )
                        t1 = wp.tile([rows, w], bf16, tag="t1", name="t1")
                        nc.vector.tensor_tensor(
                            t1[:], raw[:], cos_sb[0:rows, co : co + w], ALU.mult
                        )
                        t2 = wp.tile([rows, w], bf16, tag="t2", name="t2")
                        nc.vector.tensor_tensor(
                            t2[:], rot[:], sin_sb[0:rows, co : co + w], ALU.mult
                        )
                        for dst, r0_, r1_ in outs:
                            nc.vector.tensor_tensor(
                                dst, t1[r0_:r1_, :], t2[r0_:r1_, :], ALU.add
                            )

                    rope(kp[:], P2_sb, 128, cos2k, sin2k, c0, CH,
                         [(kTp[:, c0 : c0 + CH], 0, 128)])
                    rope(ks[:], P2_sb, HD, cos2k, sin2k, c0, CH,
                         [(kTs[:, c0 : c0 + CH], 0, HD)])

                    # V: 8 key-subtiles of 128 tokens, out [128, 195]
                    for st8 in range(8):
                        st = ch * 8 + st8
                        vps = scp.tile([128, 3 * 65], f32, tag="sc", name="vps")
                        for kt in range(5):
                            nc.tensor.matmul(
                                vps[:],
                                xch[kt][:, st8 * 128 : (st8 + 1) * 128],
                                Wv_sb[kt][:],
                                start=(kt == 0),
                                stop=(kt == 4),
                            )
                        dst = v_aug[:].rearrange("p (g s c) -> p g s c", g=3, c=65)[
                            :, :, st, :
                        ]
                        nc.vector.tensor_copy(
                            dst, vps[:].rearrange("p (g c) -> p g c", c=65)
                        )
                # ones column (col 64 of each 65-block) for the row-sum ride-along
                vones = v_aug[:].rearrange("p (n c) -> p n c", c=65)[:, :, 64:65]
                nc.vector.tensor_copy(vones, ones_sb[:, 0:48].unsqueeze(2))

                # ---- Q proj + RoPE; layout qT2 [64, j*2304 + h*256] (block-major)
                xq = []
                r0 = 0
                for kt, hk in enumerate(HK):
                    t = qtp.tile([hk, 4 * BLK], bf16, tag=f"xq{kt}", name=f"xq{kt}")
                    nc.sync.dma_start(t[:], xTq.ap()[r0 : r0 + hk, :])
                    xq.append(t)
                    r0 += hk
                # Q lives at partitions 0:64 for groups 0,2 and 64:128 for group 1
                # (matching the packed-K lhsT base so matmul bases line up).
                qT2 = qtp.tile([128, 4 * NH * BLK], bf16, tag="qT2", name="qT2")

                def qdst(h, hf):
                    # strided dst: blocks {2hf, 2hf+1}, head h
                    pb = 64 if h // 3 == 1 else 0
                    v = qT2[pb : pb + HD, :].rearrange(
                        "p (j h c) -> p j h c", j=4, h=NH
                    )
                    return v[:, 2 * hf : 2 * hf + 2, h, :]

                for hf in range(2):
                    cq0 = hf * 512
                    # 3 true pairs + packed singles (2,5) + single 8
                    packs = []  # (psum, rows, [(head, r0, r1)])
                    for pi, (h0, h1) in enumerate(((0, 1), (3, 4), (6, 7))):
                        qp = scp.tile([128, 512], f32, tag="sc", name="qp")
                        for kt in range(5):
                            nc.tensor.matmul(
                                qp[:],
                                Wq_sb[kt][:, h0 * HD : h0 * HD + 128],
                                xq[kt][:, cq0 : cq0 + 512],
                                start=(kt == 0),
                                stop=(kt == 4),
                            )
                        packs.append((qp, 128, [(h0, 0, 64), (h1, 64, 128)]))
                    qs = acp.tile([128, 512], f32, tag="acc", name="qs")
                    for half, h in ((0, 2), (64, 5)):
                        for kt in range(5):
                            nc.tensor.matmul(
                                qs[half : half + 64, :],
                                Wq_sb[kt][:, h * HD : (h + 1) * HD],
                                xq[kt][:, cq0 : cq0 + 512],
                                start=(kt == 0),
                                stop=(kt == 4),
                            )
                    packs.append((qs, 128, [(2, 0, 64), (5, 64, 128)]))
                    q8 = acp.tile([HD, 512], f32, tag="acc", name="q8")
                    for kt in range(5):
                        nc.tensor.matmul(
                            q8[:],
                            Wq_sb[kt][:, 8 * HD : 9 * HD],
                            xq[kt][:, cq0 : cq0 + 512],
                            start=(kt == 0),
                            stop=(kt == 4),
                        )
                    packs.append((q8, HD, [(8, 0, 64)]))
                    for ps, rows, heads in packs:
                        outs = [(qdst(h, hf), r0_, r1_) for h, r0_, r1_ in heads]
                        rope(ps[:], P2_sb, rows, cos2q, sin2q, cq0, 512, outs)

                # ---- attention ----
                for j in range(4):
                    ext = EXT[j]
                    mt = mskp.tile([KT, 4 * BLK], bf16, tag="msk", name="msk")
                    nc.sync.dma_start(mt[:], maskst.ap()[j, :, :])
                    cts = [
                        ctp.tile([128, BLK], bf16, tag=f"ct{t}", name=f"ct{t}")
                        for t in range(4)
                    ]
                    cts.append(ctp.tile([HD, BLK], bf16, tag="ct4", name="ct4"))
                    for g in range(NKV):
                        h0 = 3 * g
                        acc = acp.tile([65, 3 * BLK], f32, tag="acc", name="acc")
                        pb = 64 if g == 1 else 0
                        qvj = qT2[pb : pb + HD, j * NH * BLK : (j + 1) * NH * BLK]
                        for kc in range(ext):
                            ktile = (
                                kTp[g * HD : (g + 1) * HD, kc * KT : (kc + 1) * KT]
                                if g < 2
                                else kTs[:, kc * KT : (kc + 1) * KT]
                            )
                            sps = scp.tile([KT, 3 * BLK], f32, tag="sc", name="sps")
                            nc.tensor.matmul(
                                sps[:, 0:512],
                                ktile,
                                qvj[:, h0 * BLK : (h0 + 2) * BLK],
                                start=True,
                                stop=True,
                            )
                            nc.tensor.matmul(
                                sps[:, 512:768],
                                ktile,
                                qvj[:, (h0 + 2) * BLK : (h0 + 3) * BLK],
                                start=True,
                                stop=True,
                            )
                            esb = expp.tile([KT, 3 * BLK], bf16, tag="exp", name="esb")
                            nc.scalar.activation(esb[:], sps[:], AF.Exp)
                            if kc >= ext - 4:
                                off = kc - (ext - 4)
                                for i in range(3):
                                    sl = esb[:, i * BLK : (i + 1) * BLK]
                                    nc.gpsimd.tensor_tensor(
                                        sl, sl, mt[:, off * BLK : (off + 1) * BLK],
                                        ALU.mult,
                                    )
                            vt = v_aug[:, (g * 16 + kc) * 65 : (g * 16 + kc) * 65 + 65]
                            nc.tensor.matmul(
                                acc[:, 0:512], vt, esb[:, 0:512],
                                start=(kc == 0), stop=(kc == ext - 1),
                            )
                            nc.tensor.matmul(
                                acc[:, 512:768], vt, esb[:, 512:768],
                                start=(kc == 0), stop=(kc == ext - 1),
                            )
                        # normalize: 1/rowsum broadcast, scale into cts
                        rec = wp.tile([1, 3 * BLK], f32, tag="rec", name="rec")
                        nc.vector.reciprocal(rec[0:1, :], acc[64:65, :])
                        bc = wp.tile([HD, 3 * BLK], f32, tag="bc", name="bc")
                        nc.gpsimd.partition_broadcast(bc[:], rec[0:1, :])
                        for i in range(3):
                            h = h0 + i
                            t, lo = divmod(h, 2)
                            nc.vector.tensor_tensor(
                                cts[t][lo * HD : lo * HD + HD, :],
                                acc[0:HD, i * BLK : (i + 1) * BLK],
                                bc[:, i * BLK : (i + 1) * BLK],
                                ALU.mult,
                            )

                    # out projection
                    for half in range(2):
                        h0r = half * 128
                        wo = scp.tile([128, H], f32, tag="sc", name="wo")
                        for t in range(5):
                            lhsT = cts[t][:, h0r : h0r + 128]
                            nc.tensor.matmul(
                                wo[:, 0:512], lhsT, Wo_sb[t][:, 0:512],
                                start=(t == 0), stop=(t == 4),
                            )
                            nc.tensor.matmul(
                                wo[:, 512:576], lhsT, Wo_sb[t][:, 512:576],
                                start=(t == 0), stop=(t == 4),
                            )
                        osb = outp.tile([128, H], bf16, tag="osb", name="osb")
                        nc.vector.tensor_copy(osb[:], wo[:])
                        nc.sync.dma_start(
                            out.ap()[j * BLK + h0r : j * BLK + h0r + 128, :], osb[:]
                        )

            for _rep in range(reps):
                one_pass()

    nc.compile()
    return nc


def _get_nc(reps=1):
    key = f"nc{reps}"
    if key not in _CACHED:
        _CACHED[key] = _build(reps=reps)
    return _CACHED[key]


def _make_in_maps(x, cos, sin, mask, Wq, Wk, Wv, Wo):
    import ml_dtypes

    f4 = np.float32
    bf = ml_dtypes.bfloat16
    Wv65 = np.zeros((H, 3 * 65), f4)
    for g in range(3):
        Wv65[:, g * 65 : g * 65 + 64] = Wv[:, g * 64 : (g + 1) * 64]
    P2 = np.zeros((128, 128), f4)
    half = HD // 2
    for base in (0, 64):
        for m in range(half):
            P2[base + m + half, base + m] = -1.0
        for m in range(half, HD):
            P2[base + m - half, base + m] = 1.0
    cosT = np.ascontiguousarray(cos.T.astype(f4))  # [64, S]
    sinT = np.ascontiguousarray(sin.T.astype(f4))
    scale = np.float32(1.0 / np.sqrt(HD))
    maskT_full = np.ascontiguousarray(mask[0, 0].T.astype(f4))  # [k, q]
    ones48 = np.ones((128, 48), f4)

    in_maps = []
    for c in range(8):
        b = c // 2
        blocks = BLOCKS_EVEN if c % 2 == 0 else BLOCKS_ODD
        xb = x[b]  # [S, H]
        xTc = np.ascontiguousarray(xb.T.astype(f4))  # [H, S]
        qcols = np.concatenate(
            [xTc[:, blk * BLK : (blk + 1) * BLK] for blk in blocks], axis=1
        )
        cosqc = np.concatenate(
            [cosT[:, blk * BLK : (blk + 1) * BLK] for blk in blocks], axis=1
        )
        sinqc = np.concatenate(
            [sinT[:, blk * BLK : (blk + 1) * BLK] for blk in blocks], axis=1
        )
        maskstk = np.empty((4, KT, 4 * BLK), f4)
        for j, blk in enumerate(blocks):
            ext = EXT[j]
            for off in range(4):
                kc = ext - 4 + off
                sl = maskT_full[kc * KT : (kc + 1) * KT, blk * BLK : (blk + 1) * BLK]
                maskstk[j, :, off * BLK : (off + 1) * BLK] = (sl > -1.0).astype(f4)
        in_maps.append(
            {
                "xT": xTc.astype(bf),
                "xTq": np.ascontiguousarray(qcols).astype(bf),
                "Wq": (Wq.astype(f4) * scale).astype(bf),
                "Wk": Wk.astype(f4).astype(bf),
                "Wv65": Wv65.astype(bf),
                "Wo": Wo.astype(f4).astype(bf),
                "P2": P2.astype(bf),
                "cosk": cosT.astype(bf),
                "sink": sinT.astype(bf),
                "cosq": np.ascontiguousarray(cosqc).astype(bf),
                "sinq": np.ascontiguousarray(sinqc).astype(bf),
                "maskst": maskstk.astype(bf),
                "ones48": ones48.astype(bf),
            }
        )
    return in_maps


def kernel(x, cos, sin, mask, Wq, Wk, Wv, Wo, _trace=False, _trace_kwargs=None):
    from concourse import bass_utils

    x = np.asarray(x)
    in_maps = _make_in_maps(
        np.asarray(x), np.asarray(cos), np.asarray(sin), np.asarray(mask),
        np.asarray(Wq), np.asarray(Wk), np.asarray(Wv), np.asarray(Wo),
    )
    nc = _get_nc()
    kw = {}
    if _trace:
        kw["trace"] = True
        if _trace_kwargs:
            kw.update(_trace_kwargs)
    res = bass_utils.run_bass_kernel_spmd(nc, in_maps, core_ids=list(range(8)), **kw)
    out = np.empty((B, S, H), np.float32)
    for c in range(8):
        b = c // 2
        blocks = BLOCKS_EVEN if c % 2 == 0 else BLOCKS_ODD
        o = np.asarray(res.results[c]["out"]).astype(np.float32)  # [1024, 576]
        for j, blk in enumerate(blocks):
            out[b, blk * BLK : (blk + 1) * BLK, :] = o[j * BLK : (j + 1) * BLK, :]
    if _trace:
        _CACHED["last_result"] = res
    return out


# revision 19
# speedup vs baseline: 2.2273x; 1.2689x over previous
"""Distributed GQA attention kernel for Trainium2 (8 NeuronCores).

Module: B=4, S=2048, H=576, 9 Q heads / 3 KV heads, HD=64, RoPE, causal
softmax, output projection.

Sharding: core c handles batch c//2 and four 256-row query blocks
({0,3,4,7} for even c, {1,2,5,6} for odd c) -- causal work is balanced at
18 key-tile units per core. Every core computes its batch's full K/V
projection locally (duplicated across the 2 cores of a batch; cheaper
than an all-gather). One SPMD graph for all 8 cores: per-slot key-tile
extents are padded to [4,8,12,16] and the causal mask is applied from
per-core mask DATA on the last 4 key-tiles of each slot.

v2: bf16 on SBUF throughout (f32 PSUM accumulate), head-pairs packed
into 128 partitions for proj+RoPE, single [65,768] PV accumulator with
one reciprocal+broadcast per (block,group), no SBUF->SBUF DMAs
(cross-partition DVE writes), Wv stored [H,195] so PV lhsT slices need
no per-group copies, output staged bf16.
"""

import sys

if "/opt/trn_rl_repo" not in sys.path:
    sys.path.insert(0, "/opt/trn_rl_repo")

import numpy as np

B, S, H = 4, 2048, 576
NH, NKV, HD = 9, 3, 64
BLK = 256           # query block rows
KT = 128            # key tile rows
EXT = [4, 8, 12, 16]  # padded key-tile extent per block slot
BLOCKS_EVEN = [0, 3, 4, 7]
BLOCKS_ODD = [1, 2, 5, 6]
HK = [128, 128, 128, 128, 64]  # contraction tiles over H=576
CH = 1024           # kv chunk width (tokens)

_CACHED = {}


def _build(reps=1):
    from concourse import bacc, bass, mybir, tile

    f32 = mybir.dt.float32
    bf16 = mybir.dt.bfloat16
    AF = mybir.ActivationFunctionType
    ALU = mybir.AluOpType

    nc = bacc.Bacc("TRN2", target_bir_lowering=False, debug=False)

    # ---- per-core inputs (bf16 unless noted) ----
    xT = nc.dram_tensor("xT", [H, S], bf16, kind="ExternalInput")
    xTq = nc.dram_tensor("xTq", [H, 4 * BLK], bf16, kind="ExternalInput")
    Wq = nc.dram_tensor("Wq", [H, NH * HD], bf16, kind="ExternalInput")   # 1/8 folded
    Wk = nc.dram_tensor("Wk", [H, NKV * HD], bf16, kind="ExternalInput")
    Wv65 = nc.dram_tensor("Wv65", [H, 3 * 65], bf16, kind="ExternalInput")
    Wo = nc.dram_tensor("Wo", [NH * HD, H], bf16, kind="ExternalInput")
    P2 = nc.dram_tensor("P2", [128, 128], bf16, kind="ExternalInput")  # blockdiag rot
    cosk = nc.dram_tensor("cosk", [HD, S], bf16, kind="ExternalInput")
    sink = nc.dram_tensor("sink", [HD, S], bf16, kind="ExternalInput")
    cosq = nc.dram_tensor("cosq", [HD, 4 * BLK], bf16, kind="ExternalInput")
    sinq = nc.dram_tensor("sinq", [HD, 4 * BLK], bf16, kind="ExternalInput")
    maskst = nc.dram_tensor("maskst", [4, KT, 4 * BLK], bf16, kind="ExternalInput")
    ones48 = nc.dram_tensor("ones48", [128, 48], bf16, kind="ExternalInput")
    out = nc.dram_tensor("out", [4 * BLK, H], bf16, kind="ExternalOutput")

    with tile.TileContext(nc) as tc:
        with (
            tc.tile_pool(name="consts", bufs=1) as cp,
            tc.tile_pool(name="xstream", bufs=2) as xsp,
            tc.tile_pool(name="kvres", bufs=1) as kvres,
            tc.tile_pool(name="qtp", bufs=1) as qtp,
            tc.tile_pool(name="work", bufs=2) as wp,
            tc.tile_pool(name="expp", bufs=4) as expp,
            tc.tile_pool(name="mskp", bufs=4) as mskp,
            tc.tile_pool(name="ctp", bufs=2) as ctp,
            tc.tile_pool(name="outp", bufs=2) as outp,
            tc.tile_pool(name="scp", bufs=2, space="PSUM") as scp,
            tc.tile_pool(name="acp", bufs=2, space="PSUM") as acp,
        ):
            # ---- load constants ----
            def load_w(dram, cols):
                tiles = []
                r0 = 0
                for kt, hk in enumerate(HK):
                    t = cp.tile([hk, cols], bf16, tag=f"w{dram.name}{r0}",
                                name=f"w{dram.name}{r0}")
                    nc.sync.dma_start(t[:], dram.ap()[r0 : r0 + hk, :])
                    tiles.append(t)
                    r0 += hk
                return tiles

            Wk_sb = load_w(Wk, NKV * HD)
            Wv_sb = load_w(Wv65, 3 * 65)
            Wq_sb = load_w(Wq, NH * HD)
            Wo_sb = load_w(Wo, H)
            P2_sb = cp.tile([128, 128], bf16, tag="P2")
            nc.sync.dma_start(P2_sb[:], P2.ap())
            # stacked-pair cos/sin (same 64 rows twice)
            cos2k = cp.tile([128, S], bf16, tag="cos2k")
            sin2k = cp.tile([128, S], bf16, tag="sin2k")
            cos2q = cp.tile([128, 4 * BLK], bf16, tag="cos2q")
            sin2q = cp.tile([128, 4 * BLK], bf16, tag="sin2q")
            for t, d in ((cos2k, cosk), (sin2k, sink), (cos2q, cosq), (sin2q, sinq)):
                nc.sync.dma_start(t[0:64, :], d.ap())
                nc.sync.dma_start(t[64:128, :], d.ap())
            ones_sb = cp.tile([128, 48], bf16, tag="ones")
            nc.sync.dma_start(ones_sb[:], ones48.ap())
            # trigger the exp ACT-table load during the startup DMA wait
            warm = cp.tile([1, 1], f32, tag="warm")
            nc.scalar.activation(warm[:], ones_sb[0:1, 0:1], AF.Exp)

            def one_pass():
                # K layout: pair tile [128, S] = heads (g0,g1) stacked, single [64, S]
                kTp = kvres.tile([128, S], bf16, tag="kTp", name="kTp")
                kTs = kvres.tile([HD, S], bf16, tag="kTs", name="kTs")
                # V+ones layout: [128, g*1040 + st*65 + c] (3 groups x 16 subtiles x 65)
                v_aug = kvres.tile([128, 3 * 16 * 65], bf16, tag="vaug", name="v_aug")

                def rope_raw(ps, rows, w, rtag):
                    raw = wp.tile([rows, w], bf16, tag=rtag, name="raw")
                    nc.vector.tensor_copy(raw[:], ps)
                    return raw

                def rope_rot(raw, rows, w, pool, tag):
                    rot = pool.tile([rows, w], f32, tag=tag, name="rot")
                    for hf in range(0, w, 512):
                        nc.tensor.matmul(
                            rot[:, hf : hf + 512],
                            P2_sb[0:rows, 0:rows],
                            raw[:, hf : hf + 512],
                            start=True,
                            stop=True,
                        )
                    return rot

                def rope_fin(raw, rot, rows, cos_sb, sin_sb, co, w, outs):
                    t1 = wp.tile([rows, w], bf16, tag="t1", name="t1")
                    nc.vector.tensor_tensor(
                        t1[:], raw[:], cos_sb[0:rows, co : co + w], ALU.mult
                    )
                    t2 = wp.tile([rows, w], bf16, tag="t2", name="t2")
                    nc.vector.tensor_tensor(
                        t2[:], rot[:], sin_sb[0:rows, co : co + w], ALU.mult
                    )
                    for dst, r0_, r1_ in outs:
                        nc.vector.tensor_tensor(
                            dst, t1[r0_:r1_, :], t2[r0_:r1_, :], ALU.add
                        )

                # ---- K/V projection, streaming xT in 1024-token chunks ----
                def kv_chunk(ch):
                    c0 = ch * CH
                    xch = []
                    r0 = 0
                    for kt, hk in enumerate(HK):
                        t = xsp.tile([hk, CH], bf16, tag=f"xch{kt}", name=f"xch{kt}")
                        nc.sync.dma_start(t[:], xT.ap()[r0 : r0 + hk, c0 : c0 + CH])
                        xch.append(t)
                        r0 += hk
                    # K pair (g0,g1): stationary Wk[:, 0:128]
                    kp = scp.tile([128, CH], f32, tag="sc", name="kp")
                    for hf in range(2):
                        for kt in range(5):
                            nc.tensor.matmul(
                                kp[:, hf * 512 : (hf + 1) * 512],
                                Wk_sb[kt][:, 0:128],
                                xch[kt][:, hf * 512 : (hf + 1) * 512],
                                start=(kt == 0),
                                stop=(kt == 4),
                            )
                    # K single (g2): stationary Wk[:, 128:192]
                    ks = acp.tile([HD, CH], f32, tag="acc", name="ks")
                    for hf in range(2):
                        for kt in range(5):
                            nc.tensor.matmul(
                                ks[:, hf * 512 : (hf + 1) * 512],
                                Wk_sb[kt][:, 128:192],
                                xch[kt][:, hf * 512 : (hf + 1) * 512],
                                start=(kt == 0),
                                stop=(kt == 4),
                            )

                    # raw copies first (frees kp/ks PSUM slots for the V stream)
                    raw_p = rope_raw(kp[:], 128, CH, "rawp")
                    raw_s = rope_raw(ks[:], HD, CH, "raws")

                    # V: 8 key-subtiles of 128 tokens, out [128, 195]
                    for st8 in range(8):
                        st = ch * 8 + st8
                        vps = scp.tile([128, 3 * 65], f32, tag="sc", name="vps")
                        for kt in range(5):
                            nc.tensor.matmul(
                                vps[:],
                                xch[kt][:, st8 * 128 : (st8 + 1) * 128],
                                Wv_sb[kt][:],
                                start=(kt == 0),
                                stop=(kt == 4),
                            )
                        dst = v_aug[:].rearrange("p (g s c) -> p g s c", g=3, c=65)[
                            :, :, st, :
                        ]
                        nc.vector.tensor_copy(
                            dst, vps[:].rearrange("p (g c) -> p g c", c=65)
                        )

                    rot_p = rope_rot(raw_p, 128, CH, scp, "sc")
                    rot_s = rope_rot(raw_s, HD, CH, acp, "acc")
                    rope_fin(raw_p, rot_p, 128, cos2k, sin2k, c0, CH,
                             [(kTp[:, c0 : c0 + CH], 0, 128)])
                    rope_fin(raw_s, rot_s, HD, cos2k, sin2k, c0, CH,
                             [(kTs[:, c0 : c0 + CH], 0, HD)])

                # ---- Q proj + RoPE; layout qT2 [64, j*2304 + h*256] (block-major)
                # Q lives at partitions 0:64 for groups 0,2 and 64:128 for group 1
                # (matching the packed-K lhsT base so matmul bases line up).
                xq = []
                qT2 = qtp.tile([128, 4 * NH * BLK], bf16, tag="qT2", name="qT2")

                def load_xq():
                    r0 = 0
                    for kt, hk in enumerate(HK):
                        t = qtp.tile([hk, 4 * BLK], bf16, tag=f"xq{kt}", name=f"xq{kt}")
                        nc.sync.dma_start(t[:], xTq.ap()[r0 : r0 + hk, :])
                        xq.append(t)
                        r0 += hk

                def qdst(h, hf):
                    # strided dst: blocks {2hf, 2hf+1}, head h
                    pb = 64 if h // 3 == 1 else 0
                    v = qT2[pb : pb + HD, :].rearrange(
                        "p (j h c) -> p j h c", j=4, h=NH
                    )
                    return v[:, 2 * hf : 2 * hf + 2, h, :]

                def q_half(hf):
                    cq0 = hf * 512
                    # 3 true pairs + packed singles (2,5) + single 8
                    packs = []  # (raw, rows, [(head, r0, r1)], psum_pool, tag)
                    for pi, (h0, h1) in enumerate(((0, 1), (3, 4), (6, 7))):
                        qp = scp.tile([128, 512], f32, tag="sc", name="qp")
                        for kt in range(5):
                            nc.tensor.matmul(
                                qp[:],
                                Wq_sb[kt][:, h0 * HD : h0 * HD + 128],
                                xq[kt][:, cq0 : cq0 + 512],
                                start=(kt == 0),
                                stop=(kt == 4),
                            )
                        raw = rope_raw(qp[:], 128, 512, f"qr{pi}")
                        packs.append((raw, 128, [(h0, 0, 64), (h1, 64, 128)], scp, "sc"))
                    qs = acp.tile([128, 512], f32, tag="acc", name="qs")
                    for half, h in ((0, 2), (64, 5)):
                        for kt in range(5):
                            nc.tensor.matmul(
                                qs[half : half + 64, :],
                                Wq_sb[kt][:, h * HD : (h + 1) * HD],
                                xq[kt][:, cq0 : cq0 + 512],
                                start=(kt == 0),
                                stop=(kt == 4),
                            )
                    packs.append((rope_raw(qs[:], 128, 512, "qr3"), 128,
                                  [(2, 0, 64), (5, 64, 128)], acp, "acc"))
                    q8 = acp.tile([HD, 512], f32, tag="acc", name="q8")
                    for kt in range(5):
                        nc.tensor.matmul(
                            q8[:],
                            Wq_sb[kt][:, 8 * HD : 9 * HD],
                            xq[kt][:, cq0 : cq0 + 512],
                            start=(kt == 0),
                            stop=(kt == 4),
                        )
                    packs.append((rope_raw(q8[:], HD, 512, "qr4"), HD,
                                  [(8, 0, 64)], acp, "acc"))
                    rots = [
                        rope_rot(raw, rows, 512, pool, tag)
                        for raw, rows, _, pool, tag in packs
                    ]
                    for (raw, rows, heads, _, _), rot in zip(packs, rots):
                        outs = [(qdst(h, hf), r0_, r1_) for h, r0_, r1_ in heads]
                        rope_fin(raw, rot, rows, cos2q, sin2q, cq0, 512, outs)

                # interleave chunks/halves so attention on block 0 can start
                # while chunk-1 K/V and Q-half-1 still compute
                def vones(ch):
                    # ones column (col 64 of each 65-block) for the row-sum
                    dst = v_aug[:].rearrange("p (g s c) -> p g s c", g=3, c=65)[
                        :, :, ch * 8 : ch * 8 + 8, 64:65
                    ]
                    src = ones_sb[:, 0:24].rearrange(
                        "p (a b) -> p a b", a=3
                    ).unsqueeze(3)
                    nc.vector.tensor_copy(dst, src)

                kv_chunk(0)
                vones(0)
                load_xq()
                q_half(0)
                kv_chunk(1)
                vones(1)
                q_half(1)

                # mask tiles (prefetch all 4 slots)
                mts = []
                for j in range(4):
                    mt = mskp.tile([KT, 4 * BLK], bf16, tag="msk", name="msk")
                    nc.sync.dma_start(mt[:], maskst.ap()[j, :, :])
                    mts.append(mt)

                # ---- attention ----
                def emit_wo(j, cts):
                    for half in range(2):
                        h0r = half * 128
                        wo = scp.tile([128, H], f32, tag="sc", name="wo")
                        for t in range(5):
                            lhsT = cts[t][:, h0r : h0r + 128]
                            nc.tensor.matmul(
                                wo[:, 0:512], lhsT, Wo_sb[t][:, 0:512],
                                start=(t == 0), stop=(t == 4),
                            )
                            nc.tensor.matmul(
                                wo[:, 512:576], lhsT, Wo_sb[t][:, 512:576],
                                start=(t == 0), stop=(t == 4),
                            )
                        osb = outp.tile([128, H], bf16, tag="osb", name="osb")
                        nc.vector.tensor_copy(osb[:], wo[:])
                        nc.sync.dma_start(
                            out.ap()[j * BLK + h0r : j * BLK + h0r + 128, :], osb[:]
                        )

                wo_pending = None
                for j in range(4):
                    ext = EXT[j]
                    mt = mts[j]
                    cts = [
                        ctp.tile([128, BLK], bf16, tag=f"ct{t}", name=f"ct{t}")
                        for t in range(4)
                    ]
                    cts.append(ctp.tile([HD, BLK], bf16, tag="ct4", name="ct4"))
                    for g in range(NKV):
                        h0 = 3 * g
                        acc = acp.tile([65, 3 * BLK], f32, tag="acc", name="acc")
                        pb = 64 if g == 1 else 0
                        qvj = qT2[pb : pb + HD, j * NH * BLK : (j + 1) * NH * BLK]
                        for kc in range(ext):
                            ktile = (
                                kTp[g * HD : (g + 1) * HD, kc * KT : (kc + 1) * KT]
                                if g < 2
                                else kTs[:, kc * KT : (kc + 1) * KT]
                            )
                            sps = scp.tile([KT, 3 * BLK], f32, tag="sc", name="sps")
                            nc.tensor.matmul(
                                sps[:, 0:512],
                                ktile,
                                qvj[:, h0 * BLK : (h0 + 2) * BLK],
                                start=True,
                                stop=True,
                            )
                            nc.tensor.matmul(
                                sps[:, 512:768],
                                ktile,
                                qvj[:, (h0 + 2) * BLK : (h0 + 3) * BLK],
                                start=True,
                                stop=True,
                            )
                            esb = expp.tile([KT, 3 * BLK], bf16, tag="exp", name="esb")
                            nc.scalar.activation(esb[:], sps[:], AF.Exp)
                            if kc >= ext - 4:
                                off = kc - (ext - 4)
                                esv = esb[:].rearrange("p (i c) -> p i c", i=3)
                                msl = (
                                    mt[:, off * BLK : (off + 1) * BLK]
                                    .unsqueeze(1)
                                    .broadcast_to([KT, 3, BLK])
                                )
                                nc.gpsimd.tensor_tensor(esv, esv, msl, ALU.mult)
                            vt = v_aug[:, (g * 16 + kc) * 65 : (g * 16 + kc) * 65 + 65]
                            nc.tensor.matmul(
                                acc[:, 0:512], vt, esb[:, 0:512],
                                start=(kc == 0), stop=(kc == ext - 1),
                            )
                            nc.tensor.matmul(
                                acc[:, 512:768], vt, esb[:, 512:768],
                                start=(kc == 0), stop=(kc == ext - 1),
                            )
                        # normalize: 1/rowsum broadcast, scale into cts
                        rec = wp.tile([1, 3 * BLK], f32, tag="rec", name="rec")
                        nc.vector.reciprocal(rec[0:1, :], acc[64:65, :])
                        bc = wp.tile([HD, 3 * BLK], f32, tag="bc", name="bc")
                        nc.gpsimd.partition_broadcast(bc[:], rec[0:1, :])
                        for i in range(3):
                            h = h0 + i
                            t, lo = divmod(h, 2)
                            nc.vector.tensor_tensor(
                                cts[t][lo * HD : lo * HD + HD, :],
                                acc[0:HD, i * BLK : (i + 1) * BLK],
                                bc[:, i * BLK : (i + 1) * BLK],
                                ALU.mult,
                            )
                        # previous block's out-projection, emitted here so the
                        # in-order PE queue has this block's scores/PV queued
                        # ahead of Wo's cts dependency
                        if g == 0 and wo_pending is not None:
                            wo_pending()
                            wo_pending = None

                    wo_pending = (lambda jj, cc: lambda: emit_wo(jj, cc))(j, cts)
                if wo_pending is not None:
                    wo_pending()

            for _rep in range(reps):
                one_pass()

    nc.compile()
    return nc


def _get_nc(reps=1):
    key = f"nc{reps}"
    if key not in _CACHED:
        _CACHED[key] = _build(reps=reps)
    return _CACHED[key]


def _make_in_maps(x, cos, sin, mask, Wq, Wk, Wv, Wo):
    import ml_dtypes

    f4 = np.float32
    bf = ml_dtypes.bfloat16
    Wv65 = np.zeros((H, 3 * 65), f4)
    for g in range(3):
        Wv65[:, g * 65 : g * 65 + 64] = Wv[:, g * 64 : (g + 1) * 64]
    P2 = np.zeros((128, 128), f4)
    half = HD // 2
    for base in (0, 64):
        for m in range(half):
            P2[base + m + half, base + m] = -1.0
        for m in range(half, HD):
            P2[base + m - half, base + m] = 1.0
    cosT = np.ascontiguousarray(cos.T.astype(f4))  # [64, S]
    sinT = np.ascontiguousarray(sin.T.astype(f4))
    scale = np.float32(1.0 / np.sqrt(HD))
    maskT_full = np.ascontiguousarray(mask[0, 0].T.astype(f4))  # [k, q]
    ones48 = np.ones((128, 48), f4)

    in_maps = []
    for c in range(8):
        b = c // 2
        blocks = BLOCKS_EVEN if c % 2 == 0 else BLOCKS_ODD
        xb = x[b]  # [S, H]
        xTc = np.ascontiguousarray(xb.T.astype(f4))  # [H, S]
        qcols = np.concatenate(
            [xTc[:, blk * BLK : (blk + 1) * BLK] for blk in blocks], axis=1
        )
        cosqc = np.concatenate(
            [cosT[:, blk * BLK : (blk + 1) * BLK] for blk in blocks], axis=1
        )
        sinqc = np.concatenate(
            [sinT[:, blk * BLK : (blk + 1) * BLK] for blk in blocks], axis=1
        )
        maskstk = np.empty((4, KT, 4 * BLK), f4)
        for j, blk in enumerate(blocks):
            ext = EXT[j]
            for off in range(4):
                kc = ext - 4 + off
                sl = maskT_full[kc * KT : (kc + 1) * KT, blk * BLK : (blk + 1) * BLK]
                maskstk[j, :, off * BLK : (off + 1) * BLK] = (sl > -1.0).astype(f4)
        in_maps.append(
            {
                "xT": xTc.astype(bf),
                "xTq": np.ascontiguousarray(qcols).astype(bf),
                "Wq": (Wq.astype(f4) * scale).astype(bf),
                "Wk": Wk.astype(f4).astype(bf),
                "Wv65": Wv65.astype(bf),
                "Wo": Wo.astype(f4).astype(bf),
                "P2": P2.astype(bf),
                "cosk": cosT.astype(bf),
                "sink": sinT.astype(bf),
                "cosq": np.ascontiguousarray(cosqc).astype(bf),
                "sinq": np.ascontiguousarray(sinqc).astype(bf),
                "maskst": maskstk.astype(bf),
                "ones48": ones48.astype(bf),
            }
        )
    return in_maps


def kernel(x, cos, sin, mask, Wq, Wk, Wv, Wo, _trace=False, _trace_kwargs=None):
    from concourse import bass_utils

    x = np.asarray(x)
    in_maps = _make_in_maps(
        np.asarray(x), np.asarray(cos), np.asarray(sin), np.asarray(mask),
        np.asarray(Wq), np.asarray(Wk), np.asarray(Wv), np.asarray(Wo),
    )
    nc = _get_nc()
    kw = {}
    if _trace:
        kw["trace"] = True
        if _trace_kwargs:
            kw.update(_trace_kwargs)
    res = bass_utils.run_bass_kernel_spmd(nc, in_maps, core_ids=list(range(8)), **kw)
    out = np.empty((B, S, H), np.float32)
    for c in range(8):
        b = c // 2
        blocks = BLOCKS_EVEN if c % 2 == 0 else BLOCKS_ODD
        o = np.asarray(res.results[c]["out"]).astype(np.float32)  # [1024, 576]
        for j, blk in enumerate(blocks):
            out[b, blk * BLK : (blk + 1) * BLK, :] = o[j * BLK : (j + 1) * BLK, :]
    if _trace:
        _CACHED["last_result"] = res
    return out


# revision 26
# speedup vs baseline: 2.9153x; 1.3089x over previous
"""Distributed GQA attention kernel for Trainium2 (8 NeuronCores).

Module: B=4, S=2048, H=576, 9 Q heads / 3 KV heads, HD=64, RoPE, causal
softmax, output projection.

Sharding: core c handles batch c//2 and four 256-row query blocks
({0,3,4,7} for even c, {1,2,5,6} for odd c) -- causal work is balanced at
18 key-tile units per core. Every core computes its batch's full K/V
projection locally (duplicated across the 2 cores of a batch; cheaper
than an all-gather). One SPMD graph for all 8 cores: per-slot key-tile
extents are padded to [4,8,12,16] and the causal mask is applied from
per-core mask DATA on the last 4 key-tiles of each slot.

v2: bf16 on SBUF throughout (f32 PSUM accumulate), head-pairs packed
into 128 partitions for proj+RoPE, single [65,768] PV accumulator with
one reciprocal+broadcast per (block,group), no SBUF->SBUF DMAs
(cross-partition DVE writes), Wv stored [H,195] so PV lhsT slices need
no per-group copies, output staged bf16.
"""

import sys

if "/opt/trn_rl_repo" not in sys.path:
    sys.path.insert(0, "/opt/trn_rl_repo")

import numpy as np

B, S, H = 4, 2048, 576
NH, NKV, HD = 9, 3, 64
BLK = 256           # query block rows
KT = 128            # key tile rows
EXT = [4, 8, 12, 16]  # padded key-tile extent per block slot
BLOCKS_EVEN = [0, 3, 4, 7]
BLOCKS_ODD = [1, 2, 5, 6]
HK = [128, 128, 128, 128, 64]  # contraction tiles over H=576
CH = 1024           # kv chunk width (tokens)

_CACHED = {}


def _build(reps=1):
    from concourse import bacc, bass, mybir, tile

    f32 = mybir.dt.float32
    bf16 = mybir.dt.bfloat16
    AF = mybir.ActivationFunctionType
    ALU = mybir.AluOpType

    nc = bacc.Bacc("TRN2", target_bir_lowering=False, debug=False)

    # ---- per-core inputs (bf16 unless noted) ----
    # Wall columns: [0:576]=Wq (1/8 folded), [576:768]=Wk, [768:963]=Wv65,
    # [963:1539]=Wo
    WQ0, WK0, WV0, WO0 = 0, 576, 768, 963
    xT = nc.dram_tensor("xT", [H, S], bf16, kind="ExternalInput")
    xTq = nc.dram_tensor("xTq", [H, 4 * BLK], bf16, kind="ExternalInput")
    Wall = nc.dram_tensor("Wall", [H, 1539], bf16, kind="ExternalInput")
    P2 = nc.dram_tensor("P2", [128, 128], bf16, kind="ExternalInput")  # blockdiag rot
    cosk = nc.dram_tensor("cosk", [128, S], bf16, kind="ExternalInput")  # 2-stacked
    sink = nc.dram_tensor("sink", [128, S], bf16, kind="ExternalInput")
    cosq = nc.dram_tensor("cosq", [128, 4 * BLK], bf16, kind="ExternalInput")
    sinq = nc.dram_tensor("sinq", [128, 4 * BLK], bf16, kind="ExternalInput")
    maskst = nc.dram_tensor("maskst", [4, KT, 4 * BLK], bf16, kind="ExternalInput")
    out = nc.dram_tensor("out", [4 * BLK, H], bf16, kind="ExternalOutput")

    with tile.TileContext(nc) as tc:
        with (
            tc.tile_pool(name="consts", bufs=1) as cp,
            tc.tile_pool(name="xstream", bufs=2) as xsp,
            tc.tile_pool(name="kvres", bufs=1) as kvres,
            tc.tile_pool(name="qtp", bufs=1) as qtp,
            tc.tile_pool(name="work", bufs=2) as wp,
            tc.tile_pool(name="expp", bufs=4) as expp,
            tc.tile_pool(name="mskp", bufs=4) as mskp,
            tc.tile_pool(name="ctp", bufs=2) as ctp,
            tc.tile_pool(name="outp", bufs=2) as outp,
            tc.tile_pool(name="scp", bufs=2, space="PSUM") as scp,
            tc.tile_pool(name="acp", bufs=2, space="PSUM") as acp,
        ):
            # ---- load constants (5 merged weight DMAs + 4 cos/sin + P2) ----
            Wall_sb = []
            r0 = 0
            for kt, hk in enumerate(HK):
                t = cp.tile([hk, 1539], bf16, tag=f"wall{r0}", name=f"wall{r0}")
                nc.sync.dma_start(t[:], Wall.ap()[r0 : r0 + hk, :])
                Wall_sb.append(t)
                r0 += hk
            Wq_sb = [t[:, WQ0 : WQ0 + 576] for t in Wall_sb]
            Wk_sb = [t[:, WK0 : WK0 + 192] for t in Wall_sb]
            Wv_sb = [t[:, WV0 : WV0 + 195] for t in Wall_sb]
            Wo_sb = [t[:, WO0 : WO0 + 576] for t in Wall_sb]
            P2_sb = cp.tile([128, 128], bf16, tag="P2")
            nc.scalar.dma_start(P2_sb[:], P2.ap())
            # pre-stacked pair cos/sin (rows duplicated host-side)
            cos2k = cp.tile([128, S], bf16, tag="cos2k")
            sin2k = cp.tile([128, S], bf16, tag="sin2k")
            cos2q = cp.tile([128, 4 * BLK], bf16, tag="cos2q")
            sin2q = cp.tile([128, 4 * BLK], bf16, tag="sin2q")
            for t, d in ((cos2k, cosk), (sin2k, sink), (cos2q, cosq), (sin2q, sinq)):
                nc.scalar.dma_start(t[:], d.ap())
            # trigger the exp ACT-table load during the startup DMA wait
            warm = cp.tile([1, 1], f32, tag="warm")
            nc.scalar.activation(warm[:], P2_sb[0:1, 0:1], AF.Exp)

            def one_pass():
                # K layout: pair tile [128, S] = heads (g0,g1) stacked, single [64, S]
                kTp = kvres.tile([128, S], bf16, tag="kTp", name="kTp")
                kTs = kvres.tile([HD, S], bf16, tag="kTs", name="kTs")
                # V+ones layout: [128, g*1040 + st*65 + c] (3 groups x 16 subtiles x 65)
                v_aug = kvres.tile([128, 3 * 16 * 65], bf16, tag="vaug", name="v_aug")

                def rope_raw(ps, rows, w, rtag):
                    raw = wp.tile([rows, w], bf16, tag=rtag, name="raw")
                    nc.vector.tensor_copy(raw[:], ps)
                    return raw

                def rope_rot(raw, rows, w, pool, tag):
                    rot = pool.tile([rows, w], f32, tag=tag, name="rot")
                    for hf in range(0, w, 512):
                        nc.tensor.matmul(
                            rot[:, hf : hf + 512],
                            P2_sb[0:rows, 0:rows],
                            raw[:, hf : hf + 512],
                            start=True,
                            stop=True,
                        )
                    return rot

                def rope_fin(raw, rot, rows, cos_sb, sin_sb, co, w, outs):
                    t1 = wp.tile([rows, w], bf16, tag="t1", name="t1")
                    nc.vector.tensor_tensor(
                        t1[:], raw[:], cos_sb[0:rows, co : co + w], ALU.mult
                    )
                    t2 = wp.tile([rows, w], bf16, tag="t2", name="t2")
                    nc.vector.tensor_tensor(
                        t2[:], rot[:], sin_sb[0:rows, co : co + w], ALU.mult
                    )
                    for dst, r0_, r1_ in outs:
                        nc.vector.tensor_tensor(
                            dst, t1[r0_:r1_, :], t2[r0_:r1_, :], ALU.add
                        )

                # ---- K/V projection, streaming xT in 1024-token chunks ----
                def kv_chunk(ch):
                    c0 = ch * CH
                    xch = []
                    r0 = 0
                    for kt, hk in enumerate(HK):
                        t = xsp.tile([hk, CH], bf16, tag=f"xch{kt}", name=f"xch{kt}")
                        nc.sync.dma_start(t[:], xT.ap()[r0 : r0 + hk, c0 : c0 + CH])
                        xch.append(t)
                        r0 += hk
                    # K pair (g0,g1): stationary Wk[:, 0:128]
                    kp = scp.tile([128, CH], f32, tag="sc", name="kp")
                    for hf in range(2):
                        for kt in range(5):
                            nc.tensor.matmul(
                                kp[:, hf * 512 : (hf + 1) * 512],
                                Wk_sb[kt][:, 0:128],
                                xch[kt][:, hf * 512 : (hf + 1) * 512],
                                start=(kt == 0),
                                stop=(kt == 4),
                            )
                    # K single (g2): stationary Wk[:, 128:192]
                    ks = acp.tile([HD, CH], f32, tag="acc", name="ks")
                    for hf in range(2):
                        for kt in range(5):
                            nc.tensor.matmul(
                                ks[:, hf * 512 : (hf + 1) * 512],
                                Wk_sb[kt][:, 128:192],
                                xch[kt][:, hf * 512 : (hf + 1) * 512],
                                start=(kt == 0),
                                stop=(kt == 4),
                            )

                    # raw copies first (frees kp/ks PSUM slots for the V stream)
                    raw_p = rope_raw(kp[:], 128, CH, "rawp")
                    raw_s = rope_raw(ks[:], HD, CH, "raws")

                    # V: 8 key-subtiles of 128 tokens, out [128, 195]
                    for st8 in range(8):
                        st = ch * 8 + st8
                        vps = scp.tile([128, 3 * 65], f32, tag="sc", name="vps")
                        for kt in range(5):
                            nc.tensor.matmul(
                                vps[:],
                                xch[kt][:, st8 * 128 : (st8 + 1) * 128],
                                Wv_sb[kt][:],
                                start=(kt == 0),
                                stop=(kt == 4),
                            )
                        dst = v_aug[:].rearrange("p (g s c) -> p g s c", g=3, c=65)[
                            :, :, st, :
                        ]
                        nc.vector.tensor_copy(
                            dst, vps[:].rearrange("p (g c) -> p g c", c=65)
                        )

                    rot_p = rope_rot(raw_p, 128, CH, scp, "sc")
                    rot_s = rope_rot(raw_s, HD, CH, acp, "acc")
                    rope_fin(raw_p, rot_p, 128, cos2k, sin2k, c0, CH,
                             [(kTp[:, c0 : c0 + CH], 0, 128)])
                    rope_fin(raw_s, rot_s, HD, cos2k, sin2k, c0, CH,
                             [(kTs[:, c0 : c0 + CH], 0, HD)])

                # ---- Q proj + RoPE; layout qT2 [64, j*2304 + h*256] (block-major)
                # Q lives at partitions 0:64 for groups 0,2 and 64:128 for group 1
                # (matching the packed-K lhsT base so matmul bases line up).
                xq = []
                qT2 = qtp.tile([128, 4 * NH * BLK], bf16, tag="qT2", name="qT2")

                def load_xq():
                    r0 = 0
                    for kt, hk in enumerate(HK):
                        t = qtp.tile([hk, 4 * BLK], bf16, tag=f"xq{kt}", name=f"xq{kt}")
                        nc.scalar.dma_start(t[:], xTq.ap()[r0 : r0 + hk, :])
                        xq.append(t)
                        r0 += hk

                def qdst(h, hf):
                    # strided dst: blocks {2hf, 2hf+1}, head h
                    pb = 64 if h // 3 == 1 else 0
                    v = qT2[pb : pb + HD, :].rearrange(
                        "p (j h c) -> p j h c", j=4, h=NH
                    )
                    return v[:, 2 * hf : 2 * hf + 2, h, :]

                def q_half(hf):
                    cq0 = hf * 512
                    # 3 true pairs + packed singles (2,5) + single 8
                    packs = []  # (raw, rows, [(head, r0, r1)], psum_pool, tag)
                    for pi, (h0, h1) in enumerate(((0, 1), (3, 4), (6, 7))):
                        qp = scp.tile([128, 512], f32, tag="sc", name="qp")
                        for kt in range(5):
                            nc.tensor.matmul(
                                qp[:],
                                Wq_sb[kt][:, h0 * HD : h0 * HD + 128],
                                xq[kt][:, cq0 : cq0 + 512],
                                start=(kt == 0),
                                stop=(kt == 4),
                            )
                        raw = rope_raw(qp[:], 128, 512, f"qr{pi}")
                        packs.append((raw, 128, [(h0, 0, 64), (h1, 64, 128)], scp, "sc"))
                    qs = acp.tile([128, 512], f32, tag="acc", name="qs")
                    for half, h in ((0, 2), (64, 5)):
                        for kt in range(5):
                            nc.tensor.matmul(
                                qs[half : half + 64, :],
                                Wq_sb[kt][:, h * HD : (h + 1) * HD],
                                xq[kt][:, cq0 : cq0 + 512],
                                start=(kt == 0),
                                stop=(kt == 4),
                            )
                    packs.append((rope_raw(qs[:], 128, 512, "qr3"), 128,
                                  [(2, 0, 64), (5, 64, 128)], acp, "acc"))
                    q8 = acp.tile([HD, 512], f32, tag="acc", name="q8")
                    for kt in range(5):
                        nc.tensor.matmul(
                            q8[:],
                            Wq_sb[kt][:, 8 * HD : 9 * HD],
                            xq[kt][:, cq0 : cq0 + 512],
                            start=(kt == 0),
                            stop=(kt == 4),
                        )
                    packs.append((rope_raw(q8[:], HD, 512, "qr4"), HD,
                                  [(8, 0, 64)], acp, "acc"))
                    rots = [
                        rope_rot(raw, rows, 512, pool, tag)
                        for raw, rows, _, pool, tag in packs
                    ]
                    for (raw, rows, heads, _, _), rot in zip(packs, rots):
                        outs = [(qdst(h, hf), r0_, r1_) for h, r0_, r1_ in heads]
                        rope_fin(raw, rot, rows, cos2q, sin2q, cq0, 512, outs)

                # interleave chunks/halves so attention on block 0 can start
                # while chunk-1 K/V and Q-half-1 still compute
                def vones(ch):
                    # ones column (col 64 of each 65-block) for the row-sum
                    dst = v_aug[:].rearrange("p (g s c) -> p g s c", g=3, c=65)[
                        :, :, ch * 8 : ch * 8 + 8, 64:65
                    ]
                    nc.gpsimd.memset(dst, 1.0)

                kv_chunk(0)
                vones(0)
                load_xq()
                q_half(0)
                kv_chunk(1)
                vones(1)
                q_half(1)

                # mask tiles (prefetch all 4 slots)
                mts = []
                for j in range(4):
                    mt = mskp.tile([KT, 4 * BLK], bf16, tag="msk", name="msk")
                    nc.scalar.dma_start(mt[:], maskst.ap()[j, :, :])
                    mts.append(mt)

                # ---- attention ----
                def emit_wo(j, cts):
                    for half in range(2):
                        h0r = half * 128
                        wo = scp.tile([128, H], f32, tag="sc", name="wo")
                        for t in range(5):
                            lhsT = cts[t][:, h0r : h0r + 128]
                            nc.tensor.matmul(
                                wo[:, 0:512], lhsT, Wo_sb[t][:, 0:512],
                                start=(t == 0), stop=(t == 4),
                            )
                            nc.tensor.matmul(
                                wo[:, 512:576], lhsT, Wo_sb[t][:, 512:576],
                                start=(t == 0), stop=(t == 4),
                            )
                        osb = outp.tile([128, H], bf16, tag="osb", name="osb")
                        nc.vector.tensor_copy(osb[:], wo[:])
                        nc.sync.dma_start(
                            out.ap()[j * BLK + h0r : j * BLK + h0r + 128, :], osb[:]
                        )

                wo_pending = None
                for j in range(4):
                    ext = EXT[j]
                    mt = mts[j]
                    cts = [
                        ctp.tile([128, BLK], bf16, tag=f"ct{t}", name=f"ct{t}")
                        for t in range(4)
                    ]
                    cts.append(ctp.tile([HD, BLK], bf16, tag="ct4", name="ct4"))
                    for g in range(NKV):
                        h0 = 3 * g
                        acc = acp.tile([65, 3 * BLK], f32, tag="acc", name="acc")
                        pb = 64 if g == 1 else 0
                        qvj = qT2[pb : pb + HD, j * NH * BLK : (j + 1) * NH * BLK]
                        for kc in range(ext):
                            ktile = (
                                kTp[g * HD : (g + 1) * HD, kc * KT : (kc + 1) * KT]
                                if g < 2
                                else kTs[:, kc * KT : (kc + 1) * KT]
                            )
                            sps = scp.tile([KT, 3 * BLK], f32, tag="sc", name="sps")
                            nc.tensor.matmul(
                                sps[:, 0:512],
                                ktile,
                                qvj[:, h0 * BLK : (h0 + 2) * BLK],
                                start=True,
                                stop=True,
                            )
                            nc.tensor.matmul(
                                sps[:, 512:768],
                                ktile,
                                qvj[:, (h0 + 2) * BLK : (h0 + 3) * BLK],
                                start=True,
                                stop=True,
                            )
                            esb = expp.tile([KT, 3 * BLK], bf16, tag="exp", name="esb")
                            nc.scalar.activation(esb[:], sps[:], AF.Exp)
                            if kc >= ext - 4:
                                off = kc - (ext - 4)
                                esv = esb[:].rearrange("p (i c) -> p i c", i=3)
                                msl = (
                                    mt[:, off * BLK : (off + 1) * BLK]
                                    .unsqueeze(1)
                                    .broadcast_to([KT, 3, BLK])
                                )
                                nc.gpsimd.tensor_tensor(esv, esv, msl, ALU.mult)
                            vt = v_aug[:, (g * 16 + kc) * 65 : (g * 16 + kc) * 65 + 65]
                            nc.tensor.matmul(
                                acc[:, 0:512], vt, esb[:, 0:512],
                                start=(kc == 0), stop=(kc == ext - 1),
                            )
                            nc.tensor.matmul(
                                acc[:, 512:768], vt, esb[:, 512:768],
                                start=(kc == 0), stop=(kc == ext - 1),
                            )
                        # normalize: 1/rowsum broadcast, scale into cts
                        rec = wp.tile([1, 3 * BLK], f32, tag="rec", name="rec")
                        nc.vector.reciprocal(rec[0:1, :], acc[64:65, :])
                        bc = wp.tile([HD, 3 * BLK], f32, tag="bc", name="bc")
                        nc.gpsimd.partition_broadcast(bc[:], rec[0:1, :])
                        for i in range(3):
                            h = h0 + i
                            t, lo = divmod(h, 2)
                            nc.vector.tensor_tensor(
                                cts[t][lo * HD : lo * HD + HD, :],
                                acc[0:HD, i * BLK : (i + 1) * BLK],
                                bc[:, i * BLK : (i + 1) * BLK],
                                ALU.mult,
                            )
                        # previous block's out-projection, emitted here so the
                        # in-order PE queue has this block's scores/PV queued
                        # ahead of Wo's cts dependency
                        if g == 0 and wo_pending is not None:
                            wo_pending()
                            wo_pending = None

                    wo_pending = (lambda jj, cc: lambda: emit_wo(jj, cc))(j, cts)
                if wo_pending is not None:
                    wo_pending()

            for _rep in range(reps):
                one_pass()

    nc.compile()
    return nc


def _get_nc(reps=1):
    key = f"nc{reps}"
    if key not in _CACHED:
        _CACHED[key] = _build(reps=reps)
    return _CACHED[key]


def _make_in_maps(x, cos, sin, mask, Wq, Wk, Wv, Wo):
    import ml_dtypes

    f4 = np.float32
    bf = ml_dtypes.bfloat16
    Wv65 = np.zeros((H, 3 * 65), f4)
    for g in range(3):
        Wv65[:, g * 65 : g * 65 + 64] = Wv[:, g * 64 : (g + 1) * 64]
    P2 = np.zeros((128, 128), f4)
    half = HD // 2
    for base in (0, 64):
        for m in range(half):
            P2[base + m + half, base + m] = -1.0
        for m in range(half, HD):
            P2[base + m - half, base + m] = 1.0
    cosT = np.ascontiguousarray(cos.T.astype(f4))  # [64, S]
    sinT = np.ascontiguousarray(sin.T.astype(f4))
    scale = np.float32(1.0 / np.sqrt(HD))
    maskT_full = np.ascontiguousarray(mask[0, 0].T.astype(f4))  # [k, q]
    Wall = np.concatenate(
        [Wq.astype(f4) * scale, Wk.astype(f4), Wv65, Wo.astype(f4)], axis=1
    )  # [576, 1539]
    cosk2 = np.concatenate([cosT, cosT], 0)  # [128, S]
    sink2 = np.concatenate([sinT, sinT], 0)

    in_maps = []
    for c in range(8):
        b = c // 2
        blocks = BLOCKS_EVEN if c % 2 == 0 else BLOCKS_ODD
        xb = x[b]  # [S, H]
        xTc = np.ascontiguousarray(xb.T.astype(f4))  # [H, S]
        qcols = np.concatenate(
            [xTc[:, blk * BLK : (blk + 1) * BLK] for blk in blocks], axis=1
        )
        cosqc = np.concatenate(
            [cosT[:, blk * BLK : (blk + 1) * BLK] for blk in blocks], axis=1
        )
        sinqc = np.concatenate(
            [sinT[:, blk * BLK : (blk + 1) * BLK] for blk in blocks], axis=1
        )
        maskstk = np.empty((4, KT, 4 * BLK), f4)
        for j, blk in enumerate(blocks):
            ext = EXT[j]
            for off in range(4):
                kc = ext - 4 + off
                sl = maskT_full[kc * KT : (kc + 1) * KT, blk * BLK : (blk + 1) * BLK]
                maskstk[j, :, off * BLK : (off + 1) * BLK] = (sl > -1.0).astype(f4)
        in_maps.append(
            {
                "xT": xTc.astype(bf),
                "xTq": np.ascontiguousarray(qcols).astype(bf),
                "Wall": Wall.astype(bf),
                "P2": P2.astype(bf),
                "cosk": cosk2.astype(bf),
                "sink": sink2.astype(bf),
                "cosq": np.concatenate([cosqc, cosqc], 0).astype(bf),
                "sinq": np.concatenate([sinqc, sinqc], 0).astype(bf),
                "maskst": maskstk.astype(bf),
            }
        )
    return in_maps


def kernel(x, cos, sin, mask, Wq, Wk, Wv, Wo, _trace=False, _trace_kwargs=None):
    from concourse import bass_utils

    x = np.asarray(x)
    in_maps = _make_in_maps(
        np.asarray(x), np.asarray(cos), np.asarray(sin), np.asarray(mask),
        np.asarray(Wq), np.asarray(Wk), np.asarray(Wv), np.asarray(Wo),
    )
    nc = _get_nc()
    kw = {}
    if _trace:
        kw["trace"] = True
        if _trace_kwargs:
            kw.update(_trace_kwargs)
    res = bass_utils.run_bass_kernel_spmd(nc, in_maps, core_ids=list(range(8)), **kw)
    out = np.empty((B, S, H), np.float32)
    for c in range(8):
        b = c // 2
        blocks = BLOCKS_EVEN if c % 2 == 0 else BLOCKS_ODD
        o = np.asarray(res.results[c]["out"]).astype(np.float32)  # [1024, 576]
        for j, blk in enumerate(blocks):
            out[b, blk * BLK : (blk + 1) * BLK, :] = o[j * BLK : (j + 1) * BLK, :]
    if _trace:
        _CACHED["last_result"] = res
    return out
